# revision 1
# baseline (speedup 1.0000x reference)
"""Trainium2 Bass kernel for an 8-head MultiHeadAttention (B=2, S=4096, H=512).

Sharding: 8 NeuronCores, each takes (one batch, two heads):
    core c -> batch b = c // 4, heads {2*(c%4), 2*(c%4)+1}.

Per-core pipeline (mixed precision, validated ~4e-3 scale-relative absmax
against the fp32 reference -- see test.py):
  - Host pre-transposes x[b] -> xT [512, 4096] (rounded to fp32r = e8m11)
    and slices weight columns for the core's two heads.
  - q/k/v projections all run as fp32r matmuls (N=512, kf-outer so PE
    starts on the first 2MB DMA chunk) in head-transposed layout
    [128 rows = 2 heads x 64 dims, S]; PSUM evictions cast to bf16 with
    the bias fused (tensor_scalar_add).
  - v is then moved to natural layout [S, 128] by 64 hardware DMA
    transposes (bf16 X-bar path, off the critical engines), with a ones
    column appended per head so the attention matmul also accumulates the
    softmax denominator for free.
  - scoresT = kT.T @ qT per head: two row-tiled concurrent bf16 matmuls
    (K=64 in rows 0-63 / 64-127) into one 2-bank PSUM tile.
  - One big Exp activation per k-chunk ([128, 1024], scale 1/8 folded in,
    no max-subtraction: scores are provably small here) -> bf16.
  - attn@v: bf16 matmuls accumulating outT' [65, 512] in PSUM
    (row 64 = denominators).
  - Normalize: denominator row cast to f32r, K=1 ones-matmul broadcasts it
    across partitions, reciprocal_approx_fast + multiply on DVE.
  - Output written in transposed layout outT [128, S]; host reassembles.
"""

import os
import sys

sys.path.insert(0, "/opt/trn_rl_repo")

import ml_dtypes
import numpy as np

import concourse.bass as bass  # noqa: E402
import concourse.tile as tile  # noqa: E402
from concourse import bacc, mybir  # noqa: E402
from concourse.bass_utils import run_bass_kernel_spmd  # noqa: E402

B, S, H = 2, 4096, 512
NH, HD = 8, 64
NCORES = 8
HPC = 2  # heads per core
DPC = HPC * HD  # head dims per core = 128
P = 128  # partitions
QB = 512  # query block (matmul free dim)
KC = 128  # key chunk (contraction tile)
KF = H // P  # feature chunks for projections = 4
NKC = S // KC  # 32
NQB = S // QB  # 8
VPAD = 80  # padded per-(kc,h) v row (64 v + ones + align padding)

f32 = mybir.dt.float32
f32r = mybir.dt.float32r
bf16 = mybir.dt.bfloat16
_np_bf16 = ml_dtypes.bfloat16


def _emit_kernel(ctx, tc, outT, xT, wq, wk, wv, bias3, ones, onescol):
    nc = tc.nc

    const = ctx.enter_context(tc.tile_pool(name="const", bufs=1))

    # ---- weights/constants first (small), then x: PE unblocks early ----
    wq_sb = const.tile([P, KF, DPC], f32r)
    wk_sb = const.tile([P, KF, DPC], f32r)
    wv_sb = const.tile([P, KF, DPC], f32r)
    for w_sb, w in ((wk_sb, wk), (wv_sb, wv), (wq_sb, wq)):
        nc.sync.dma_start(
            out=w_sb[:], in_=w.rearrange("(kf p) m -> p kf m", p=P)
        )
    # biases [3, 128] -> sbuf [128, 3] (partition = output dim; q, k, v)
    bias_sb = const.tile([P, 3], f32)
    nc.sync.dma_start(out=bias_sb[:], in_=bias3.rearrange("a m -> m a"))
    ones_sb = const.tile([1, P], f32r)
    nc.sync.dma_start(out=ones_sb[:], in_=ones[:])

    # xT [H, S] -> sbuf [128, KF, S] (partition = feature % 128);
    # 1MB half-chunks so the first wave's matmuls unblock sooner
    xT_sb = const.tile([P, KF, S], f32r)
    for kf in range(KF):
        for hh in range(2):
            nc.sync.dma_start(
                out=xT_sb[:, kf, hh * (S // 2) : (hh + 1) * (S // 2)],
                in_=xT[kf * P : (kf + 1) * P, hh * (S // 2) : (hh + 1) * (S // 2)],
            )

    # ---- projections: q/k/v in T layout, fp32r matmuls, bf16 evictions ----
    qkT_sb = const.tile([P, 2, S], bf16)
    vT_sb = const.tile([P, S], bf16)
    # v natural + ones column: vp_sb[p, kc, h, :64] = v, [..., 64] = 1
    vp_sb = const.tile([P, NKC, HPC, VPAD], bf16)
    nc.sync.dma_start(out=vp_sb[:, :, :, HD : HD + 1], in_=onescol[:])

    with tc.tile_pool(name="proj_psum", bufs=8, space="PSUM") as pp:
        with nc.named_scope("proj"):
            for proj, w_sb in ((1, wk_sb), (2, wv_sb), (0, wq_sb)):
                pss = [
                    pp.tile([P, QB], f32, tag="ps", name=f"pj{proj}_{sb}")
                    for sb in range(S // QB)
                ]
                # kf-outer: the first 8 matmuls need only xT chunk 0
                for kf in range(KF):
                    for sb in range(S // QB):
                        nc.tensor.matmul(
                            pss[sb][:],
                            lhsT=w_sb[:, kf, :],
                            rhs=xT_sb[:, kf, sb * QB : (sb + 1) * QB],
                            start=(kf == 0),
                            stop=(kf == KF - 1),
                        )
                for sb in range(S // QB):
                    dst = (
                        vT_sb[:, sb * QB : (sb + 1) * QB]
                        if proj == 2
                        else qkT_sb[:, proj, sb * QB : (sb + 1) * QB]
                    )
                    # psum -> sbuf eviction, fused bias add, bf16 out
                    with nc.allow_low_precision(reason="bf16 attention"):
                        nc.vector.tensor_scalar_add(
                            dst, pss[sb][:], bias_sb[:, proj : proj + 1]
                        )
                if proj == 2:
                    # v: T layout -> natural via hardware DMA transpose
                    # (X-bar, bf16), one per head: in [64, S] -> out
                    # [128, NKC, 64]. The v-wave runs after the k-wave,
                    # which gates on the last xT chunk, so all input DMAs
                    # have drained; the transposes overlap the q-wave.
                    # (Finer-grained splits that overlap the eviction
                    # stream hard-crash the device - do not pipeline these.)
                    for h in range(HPC):
                        nc.sync.dma_start_transpose(
                            out=vp_sb[:, :, h, 0:HD],
                            in_=vT_sb[h * HD : (h + 1) * HD, :],
                        )

    # ---- attention ----
    sc_pool = ctx.enter_context(tc.tile_pool(name="sc", bufs=2, space="PSUM"))
    ot_pool = ctx.enter_context(tc.tile_pool(name="ot", bufs=3, space="PSUM"))
    rb_pool = ctx.enter_context(tc.tile_pool(name="rb", bufs=1, space="PSUM"))
    ex_pool = ctx.enter_context(tc.tile_pool(name="ex", bufs=3))
    fin_pool = ctx.enter_context(tc.tile_pool(name="fin", bufs=4))
    rc_pool = ctx.enter_context(tc.tile_pool(name="rc", bufs=4))
    res_pool = ctx.enter_context(tc.tile_pool(name="res", bufs=4))

    with nc.named_scope("attn"):
        for qb in range(NQB):
            q0, q1 = qb * QB, (qb + 1) * QB
            oT = [
                ot_pool.tile([HD + 1, QB], f32, tag="oT", name=f"oT{qb}_{h}")
                for h in range(HPC)
            ]
            for kc in range(NKC):
                sc = sc_pool.tile([P, HPC, QB], f32, tag="sc")
                for h in range(HPC):
                    # scoresT[k, q] for head h; K = 64, rows 64h..64h+63
                    nc.tensor.matmul(
                        sc[:, h, :],
                        lhsT=qkT_sb[
                            h * HD : (h + 1) * HD, 1, kc * KC : (kc + 1) * KC
                        ],
                        rhs=qkT_sb[h * HD : (h + 1) * HD, 0, q0:q1],
                        start=True,
                        stop=True,
                        tile_position=(h * HD, 0),
                    )
                ex = ex_pool.tile([P, HPC, QB], bf16, tag="ex")
                nc.scalar.activation(
                    ex[:],
                    sc[:],
                    mybir.ActivationFunctionType.Exp,
                    scale=1.0 / np.sqrt(HD),
                )
                for h in range(HPC):
                    nc.tensor.matmul(
                        oT[h][:],
                        lhsT=vp_sb[:, kc, h, 0 : HD + 1],
                        rhs=ex[:, h, :],
                        start=(kc == 0),
                        stop=(kc == NKC - 1),
                    )
            for h in range(HPC):
                # sums row (f32r) straight from PSUM so the broadcast matmul
                # only waits on this one cheap DVE op
                srow = rc_pool.tile([1, QB], f32r, tag="srow", name=f"sr{qb}_{h}")
                with nc.allow_low_precision(reason="f32r sums, 2^-12 rel"):
                    nc.vector.tensor_copy(srow[:], oT[h][HD : HD + 1, :])
                fin = fin_pool.tile([HD + 1, QB], f32, tag="fin")
                nc.vector.tensor_copy(fin[:], oT[h][:])
                rb = rb_pool.tile([HD, QB], f32, tag="rb")
                nc.tensor.matmul(
                    rb[:],
                    lhsT=ones_sb[:, :HD],
                    rhs=srow[:],
                    start=True,
                    stop=True,
                )
                rcb = res_pool.tile([HD, QB], f32, tag="rcb", name=f"rcb{qb}_{h}")
                nc.vector.reciprocal_approx_fast(out=rcb[:], in_=rb[:])
                res = res_pool.tile([HD, QB], f32, tag="res")
                nc.vector.tensor_mul(res[:], fin[:HD, :], rcb[:])
                nc.sync.dma_start(
                    out=outT[h * HD : (h + 1) * HD, q0:q1], in_=res[:]
                )


def build_nc():
    from contextlib import ExitStack

    nc = bacc.Bacc(
        "TRN2",
        target_bir_lowering=False,
        debug=False,
        num_devices=NCORES,
    )
    xT = nc.dram_tensor("xT", [H, S], f32r, kind="ExternalInput").ap()
    wq = nc.dram_tensor("wq", [H, DPC], f32r, kind="ExternalInput").ap()
    wk = nc.dram_tensor("wk", [H, DPC], f32r, kind="ExternalInput").ap()
    wv = nc.dram_tensor("wv", [H, DPC], f32r, kind="ExternalInput").ap()
    bias3 = nc.dram_tensor("bias3", [3, DPC], f32, kind="ExternalInput").ap()
    ones = nc.dram_tensor("ones", [1, P], f32r, kind="ExternalInput").ap()
    onescol = nc.dram_tensor(
        "onescol", [P, NKC * HPC], bf16, kind="ExternalInput"
    ).ap()
    outT = nc.dram_tensor("outT", [DPC, S], f32, kind="ExternalOutput").ap()
    with tile.TileContext(nc) as tc, ExitStack() as ctx:
        _emit_kernel(ctx, tc, outT, xT, wq, wk, wv, bias3, ones, onescol)
    nc.compile()
    return nc


_NC_CACHE = None


def _get_nc():
    global _NC_CACHE
    if _NC_CACHE is None:
        _NC_CACHE = build_nc()
    return _NC_CACHE


def _round_f32r(a):
    """Round fp32 -> fp32r (e8m11: low 12 mantissa bits zeroed, RNE).

    The PE consumes fp32r operands by their top 20 bits; pre-rounding on
    the host matches what the hardware would use."""
    b = np.ascontiguousarray(a, dtype=np.float32).view(np.uint32)
    t = b + np.uint32(0x7FF) + ((b >> np.uint32(12)) & np.uint32(1))
    return (t & np.uint32(0xFFFFF000)).view(np.float32)


def _shard_inputs(x, Wq, bq, Wk, bk, Wv, bv):
    """Build per-core input maps (host does layout only: transpose/slice)."""
    x = np.ascontiguousarray(np.asarray(x, dtype=np.float32))
    in_maps = []
    xT_by_batch = [_round_f32r(x[b].T) for b in range(B)]
    for c in range(NCORES):
        b, p = c // (NCORES // B), c % (NCORES // B)
        cols = slice(p * DPC, (p + 1) * DPC)
        in_maps.append(
            {
                "xT": xT_by_batch[b],
                "wq": _round_f32r(np.asarray(Wq, np.float32)[:, cols]),
                "wk": _round_f32r(np.asarray(Wk, np.float32)[:, cols]),
                "wv": _round_f32r(np.asarray(Wv, np.float32)[:, cols]),
                "bias3": np.stack(
                    [
                        np.asarray(bq, np.float32)[cols],
                        np.asarray(bk, np.float32)[cols],
                        np.asarray(bv, np.float32)[cols],
                    ]
                ),
                "ones": np.ones((1, P), dtype=np.float32),
                "onescol": np.ones((P, NKC * HPC), dtype=_np_bf16),
            }
        )
    return in_maps


def _assemble(results):
    out = np.empty((B, S, H), dtype=np.float32)
    for c in range(NCORES):
        b, p = c // (NCORES // B), c % (NCORES // B)
        outT = results[c]["outT"]  # [128, S]
        out[b, :, p * DPC : (p + 1) * DPC] = outT.T
    return out


def run(inputs, trace=False):
    nc = _get_nc()
    in_maps = _shard_inputs(**inputs)
    res = run_bass_kernel_spmd(nc, in_maps, list(range(NCORES)), trace=trace)
    return _assemble(res.results), res


def kernel(**inputs):
    out, _ = run(inputs)
    return out

